# revision 37
# baseline (speedup 1.0000x reference)
"""GATv2 (3-layer, PyG semantics) + global mean pool + MLP on 8 trn2 NeuronCores.

Self-contained: hardcodes problem shapes from nn_GAT_47906065219807.
Sharding: data-parallel over contiguous node ranges (2500 nodes/core); each
core owns edges whose dst lands in its range (edges cross shards via an
AllGather of the source-side projections).

v5:
  - gr expanded from the xr window tile by a PE matmul with a transposed
    edge mask; gx added in-PSUM via an identity matmul; ACT applies Prelu
    straight from PSUM (no DVE add, no gr gather).
  - softmax weights folded into the aggregation matmul's stationary mask
    (alphaM = emask * ex, one broadcast DVE op per tile); division by the
    softmax denominator happens per-node after aggregation.
  - att-dot via fused per-head scalar_tensor_tensor accumulate.
  - each layer's AllGather is split into two half-node collectives: AG-A
    fires halfway through the xl projection and AG-B right after it, so
    both overlap the remaining projection work. Edges are bucketed per
    (window, src-half) so each window runs two gathers, one per half.
"""
import numpy as np
import ml_dtypes

import concourse.bacc as bacc
import concourse.mybir as mybir
import concourse.tile as tile
from concourse.bass_utils import run_bass_kernel_spmd

# problem constants
N_NODES = 20000
N_EDGES = 120000
N_GRAPHS = 512
F_IN = 300
NHID = 256
NOUT = 768
SLOPE = 0.2
EPS = 1e-16

NCORES = 8
NLOC = N_NODES // NCORES            # 2500
NPAD = 2560                         # 20 tiles of 128
NWIN = NPAD // 128                  # 20 windows / node tiles per core
KIN_PAD = 384                       # F_IN padded to 3*128
NHALF = NPAD // 2                   # 1280 rows per AllGather half

# per-layer dims: (K_in_padded, F_out, heads, concat)
LAYERS = [
    (KIN_PAD, 4 * NHID, 4, True),
    (4 * NHID, 4 * NHID, 4, True),
    (4 * NHID, 6 * NHID, 6, False),
]

_BF = ml_dtypes.bfloat16
_PROG_CACHE = {}
SIM_SAFE = False      # sim lacks the fused DVE-accumulator writeback
STT_SCORES = True     # fused per-head stt+accum scores (else mult+3D-reduce)
BCAST_ALPHAM = True   # single broadcast-TT alphaM per tile (else per-head)


def _bf16(a):
    return np.ascontiguousarray(a.astype(_BF)).view(np.uint16)


def _wrap_idx(flat_idx):
    """dma_gather index layout: slot i -> [i % 16, i // 16], replicated to
    128 partitions (8 Q7 cores x 16)."""
    n = flat_idx.shape[0]
    assert n % 16 == 0
    w = flat_idx.reshape(n // 16, 16).T.astype(np.int16)
    return np.tile(w, (8, 1)).copy()


def _preprocess(inputs):
    """Host-side sharding/layout. Returns (key, in_maps)."""
    x = np.asarray(inputs["x"], np.float32)
    ei = np.asarray(inputs["edge_index"]).astype(np.int64)
    batch = np.asarray(inputs["batch"]).astype(np.int64)

    loops = np.arange(N_NODES, dtype=np.int64)
    src = np.concatenate([ei[0], loops])
    dst = np.concatenate([ei[1], loops])

    src_loc = src % NLOC
    src_core = src // NLOC
    src_half = (src_loc >= NHALF).astype(np.int64)          # 0 = A, 1 = B
    # row index within the half's AllGather output [NCORES*NHALF, F]
    src_row = src_core * NHALF + (src_loc - src_half * NHALF)

    core_of = dst // NLOC
    # per (core, window, half) edge buckets
    buckets = [[[[], []] for _ in range(NWIN)] for _ in range(NCORES)]
    order = np.argsort(dst, kind="stable")
    for e in order:
        c = core_of[e]
        dl = dst[e] - c * NLOC
        buckets[c][dl // 128][src_half[e]].append(e)
    TwA = tuple(
        max((len(buckets[c][w][0]) + 127) // 128 for c in range(NCORES))
        for w in range(NWIN)
    )
    TwB = tuple(
        max((len(buckets[c][w][1]) + 127) // 128 for c in range(NCORES))
        for w in range(NWIN)
    )
    # concatenated slot space per window: A tiles then B tiles
    Tw = tuple(a + b for a, b in zip(TwA, TwB))
    Soff = np.concatenate([[0], np.cumsum([t * 128 for t in Tw])])
    SoffA = np.concatenate([[0], np.cumsum([t * 128 for t in TwA])])
    SoffB = np.concatenate([[0], np.cumsum([t * 128 for t in TwB])])

    cnt = np.bincount(batch, minlength=N_GRAPHS).astype(np.float32)
    rcnt = 1.0 / np.maximum(cnt, 1.0)

    def wT_pad(w, kpad):
        wt = w.T.astype(np.float32)                    # [K, F]
        K, F = wt.shape
        out = np.zeros((kpad, F), np.float32)
        out[:K] = wt
        return _bf16(out.reshape(kpad // 128, 128, F).transpose(1, 0, 2)
                     .reshape(128, (kpad // 128) * F))

    shared = {
        "w1l": wT_pad(np.asarray(inputs["c1_wl"]), KIN_PAD),
        "w1r": wT_pad(np.asarray(inputs["c1_wr"]), KIN_PAD),
        "w2l": wT_pad(np.asarray(inputs["c2_wl"]), 4 * NHID),
        "w2r": wT_pad(np.asarray(inputs["c2_wr"]), 4 * NHID),
        "w3l": wT_pad(np.asarray(inputs["c3_wl"]), 4 * NHID),
        "w3r": wT_pad(np.asarray(inputs["c3_wr"]), 4 * NHID),
        "att1": _bf16(np.tile(np.asarray(inputs["c1_att"]).reshape(1, -1), (128, 1))),
        "att2": _bf16(np.tile(np.asarray(inputs["c2_att"]).reshape(1, -1), (128, 1))),
        "att3": _bf16(np.tile(np.asarray(inputs["c3_att"]).reshape(1, -1), (128, 1))),
        "b1": np.tile(np.asarray(inputs["c1_b"], np.float32).reshape(1, -1), (128, 1)),
        "b2": np.tile(np.asarray(inputs["c2_b"], np.float32).reshape(1, -1), (128, 1)),
        "b3": np.tile(np.asarray(inputs["c3_b"], np.float32).reshape(1, -1), (128, 1)),
        "rcnt": np.tile(rcnt.reshape(1, -1), (128, 1)).astype(np.float32),
        "wfc1": wT_pad(np.asarray(inputs["fp1_w"]), 256),
        "wfc2": wT_pad(np.asarray(inputs["fp2_w"]), 256),
        "bfc1": np.asarray(inputs["fp1_b"], np.float32).reshape(2, 128).T.copy(),
        "bfc2": np.tile(np.asarray(inputs["fp2_b"], np.float32).reshape(1, -1),
                        (128, 1)),
        "ident": _bf16(np.eye(128, dtype=np.float32)),
    }

    in_maps = []
    for c in range(NCORES):
        xc = np.zeros((NPAD, KIN_PAD), np.float32)
        xc[:NLOC, :F_IN] = x[c * NLOC:(c + 1) * NLOC]
        xT = xc.T.reshape(KIN_PAD // 128, 128, NPAD).transpose(1, 0, 2)
        xT = _bf16(xT.reshape(128, (KIN_PAD // 128) * NPAD))

        tot = int(Soff[-1])
        isrcA = np.zeros(int(SoffA[-1]), np.int64)
        isrcB = np.zeros(int(SoffB[-1]), np.int64)
        emask = np.zeros((128, tot), np.float32)
        emaskT = np.zeros((128, tot), np.float32)
        for w in range(NWIN):
            s0 = int(Soff[w])
            for hf, (isrc_h, Soff_h, toff) in enumerate(
                    ((isrcA, SoffA, 0), (isrcB, SoffB, TwA[w]))):
            # half A occupies tiles [0, TwA), half B tiles [TwA, TwA+TwB)
                es = buckets[c][w][hf]
                sh0 = int(Soff_h[w])
                for i, e in enumerate(es):
                    isrc_h[sh0 + i] = src_row[e]
                    n = (dst[e] - c * NLOC) - w * 128
                    col = s0 + (toff + i // 128) * 128
                    emask[i % 128, col + n] = 1.0
                    emaskT[n, col + (i % 128)] = 1.0
        pmask = np.zeros((128, NWIN * N_GRAPHS), np.float32)
        bl = batch[c * NLOC:(c + 1) * NLOC]
        for nl in range(NLOC):
            pmask[nl % 128, (nl // 128) * N_GRAPHS + bl[nl]] = 1.0

        m = dict(shared)
        m["xT"] = xT
        m["isrcA"] = _wrap_idx(isrcA)
        m["isrcB"] = _wrap_idx(isrcB)
        m["emask"] = _bf16(emask)
        m["emaskT"] = _bf16(emaskT)
        m["pmask"] = _bf16(pmask)
        in_maps.append(m)
    return (TwA, TwB), in_maps


def _build(key):
    TwA, TwB = key
    Tw = tuple(a + b for a, b in zip(TwA, TwB))
    TMAX = max(Tw)
    Soff = [0]
    SoffA = [0]
    SoffB = [0]
    for a, b in zip(TwA, TwB):
        Soff.append(Soff[-1] + (a + b) * 128)
        SoffA.append(SoffA[-1] + a * 128)
        SoffB.append(SoffB[-1] + b * 128)
    TOT, TOTA, TOTB = Soff[-1], SoffA[-1], SoffB[-1]
    nc = bacc.Bacc("TRN2", target_bir_lowering=False, debug=False,
                   num_devices=NCORES)
    dt = mybir.dt
    AF = mybir.ActivationFunctionType
    OP = mybir.AluOpType

    def inp(name, shape, d):
        return nc.dram_tensor(name, shape, d, kind="ExternalInput")

    xT_in = inp("xT", [128, (KIN_PAD // 128) * NPAD], dt.bfloat16)
    isrcA_in = inp("isrcA", [128, TOTA // 16], dt.int16)
    isrcB_in = inp("isrcB", [128, TOTB // 16], dt.int16)
    emask_in = inp("emask", [128, TOT], dt.bfloat16)
    emaskT_in = inp("emaskT", [128, TOT], dt.bfloat16)
    pmask_in = inp("pmask", [128, NWIN * N_GRAPHS], dt.bfloat16)
    ident_in = inp("ident", [128, 128], dt.bfloat16)
    w_in = [(inp("w1l", [128, 3 * 1024], dt.bfloat16),
             inp("w1r", [128, 3 * 1024], dt.bfloat16)),
            (inp("w2l", [128, 8 * 1024], dt.bfloat16),
             inp("w2r", [128, 8 * 1024], dt.bfloat16)),
            (inp("w3l", [128, 8 * 1536], dt.bfloat16),
             inp("w3r", [128, 8 * 1536], dt.bfloat16))]
    att_in = [inp("att1", [128, 1024], dt.bfloat16),
              inp("att2", [128, 1024], dt.bfloat16),
              inp("att3", [128, 1536], dt.bfloat16)]
    b_in = [inp("b1", [128, 1024], dt.float32),
            inp("b2", [128, 1024], dt.float32),
            inp("b3", [128, 256], dt.float32)]
    rcnt_in = inp("rcnt", [128, N_GRAPHS], dt.float32)
    wfc1_in = inp("wfc1", [128, 2 * 256], dt.bfloat16)
    wfc2_in = inp("wfc2", [128, 2 * 768], dt.bfloat16)
    bfc1_in = inp("bfc1", [128, 2], dt.float32)
    bfc2_in = inp("bfc2", [128, 768], dt.float32)
    out_ext = nc.dram_tensor("out", [N_GRAPHS, NOUT], dt.float32,
                             kind="ExternalOutput")

    # internal DRAM
    xl_loc = [nc.dram_tensor(f"xl_loc{l}", [NPAD, F], dt.bfloat16)
              for l, (_, F, _, _) in enumerate(LAYERS)]
    xr_loc = [nc.dram_tensor(f"xr_loc{l}", [NPAD, F], dt.bfloat16)
              for l, (_, F, _, _) in enumerate(LAYERS)]
    xl_fullA = [nc.dram_tensor(f"xl_fullA{l}", [NCORES * NHALF, F],
                               dt.bfloat16, addr_space="Shared")
                for l, (_, F, _, _) in enumerate(LAYERS)]
    xl_fullB = [nc.dram_tensor(f"xl_fullB{l}", [NCORES * NHALF, F],
                               dt.bfloat16, addr_space="Shared")
                for l, (_, F, _, _) in enumerate(LAYERS)]
    h_dram = [nc.dram_tensor(f"h_dram{l}", [NPAD, 1024], dt.bfloat16)
              for l in range(2)]
    pool_loc = nc.dram_tensor("pool_loc", [256, N_GRAPHS], dt.float32)
    pool_full = nc.dram_tensor("pool_full", [256, N_GRAPHS], dt.float32,
                               addr_space="Shared")

    rg = [list(range(NCORES))]

    with tile.TileContext(nc) as tc:
        with (
            tc.tile_pool(name="persist", bufs=1) as ppool,
        ):
            isrcA_t = ppool.tile([128, TOTA // 16], dt.int16)
            nc.sync.dma_start(out=isrcA_t[:, :], in_=isrcA_in[:, :])
            isrcB_t = ppool.tile([128, TOTB // 16], dt.int16)
            nc.sync.dma_start(out=isrcB_t[:, :], in_=isrcB_in[:, :])
            ident_t = ppool.tile([128, 128], dt.bfloat16)
            nc.sync.dma_start(out=ident_t[:, :], in_=ident_in[:, :])
            zeros_t = ppool.tile([128, 1024], dt.bfloat16)
            nc.vector.memset(zeros_t[:, :], 0.0)

            for l, (K, F, H, concat) in enumerate(LAYERS):
                KB = K // 128
                NCH = F // 512

                aux_cm = tc.tile_pool(name=f"aux{l}", bufs=1)
                auxpool = aux_cm.__enter__()
                att_t = auxpool.tile([128, F], dt.bfloat16)
                nc.sync.dma_start(out=att_t[:, :], in_=att_in[l][:, :])
                bias_t = auxpool.tile([128, F if concat else 256],
                                      dt.float32)
                nc.sync.dma_start(out=bias_t[:, :], in_=b_in[l][:, :])
                if l == 2:
                    pmask_t = auxpool.tile([128, NWIN * N_GRAPHS],
                                           dt.bfloat16)
                    nc.sync.dma_start(out=pmask_t[:, :], in_=pmask_in[:, :])

                # ---------------- projections ----------------
                with (
                    tc.tile_pool(name=f"pj{l}", bufs=1) as pjpool,
                    tc.tile_pool(name=f"mm{l}", bufs=4) as mmpool,
                    tc.tile_pool(name=f"psA{l}", bufs=3, space="PSUM") as psA,
                ):
                    hT = pjpool.tile([128, KB, NPAD], dt.bfloat16, tag="hT")
                    if l == 0:
                        for b in range(KB):
                            nc.sync.dma_start(
                                out=hT[:, b, :],
                                in_=xT_in[:, b * NPAD:(b + 1) * NPAD])
                    else:
                        QN = NPAD // 4
                        for q in range(4):
                            for b in range(KB):
                                nc.sync.dma_start(
                                    out=hT[:, b, q * QN:(q + 1) * QN],
                                    in_=h_dram[l - 1][q * QN:(q + 1) * QN,
                                                      b * 128:(b + 1) * 128],
                                    transpose=True)
                    wl_t = pjpool.tile([128, KB, F], dt.bfloat16)
                    wr_t = pjpool.tile([128, KB, F], dt.bfloat16)
                    for wt, win in ((wl_t, w_in[l][0]), (wr_t, w_in[l][1])):
                        for b in range(KB):
                            nc.sync.dma_start(out=wt[:, b, :],
                                              in_=win[:, b * F:(b + 1) * F])
                    for side, (wt, dst_dram) in enumerate(
                            ((wl_t, xl_loc[l]), (wr_t, xr_loc[l]))):
                        for t in range(NWIN):
                            for ch in range(NCH):
                                ps = psA.tile([128, 512], dt.float32,
                                              tag="mmps")
                                for b in range(KB):
                                    nc.tensor.matmul(
                                        ps[:, :],
                                        hT[:, b, t * 128:(t + 1) * 128],
                                        wt[:, b, ch * 512:(ch + 1) * 512],
                                        start=(b == 0), stop=(b == KB - 1))
                                ob = mmpool.tile([128, 512], dt.bfloat16,
                                                 tag="mmout")
                                nc.scalar.copy(ob[:, :], ps[:, :])
                                nc.sync.dma_start(
                                    out=dst_dram[t * 128:(t + 1) * 128,
                                                 ch * 512:(ch + 1) * 512],
                                    in_=ob[:, :])
                            if side == 0 and t == NWIN // 2 - 1:
                                nc.gpsimd.collective_compute(
                                    "AllGather", mybir.AluOpType.bypass,
                                    replica_groups=rg,
                                    ins=[xl_loc[l].ap()[0:NHALF, :].opt()],
                                    outs=[xl_fullA[l].ap().opt()])
                        if side == 0:
                            nc.gpsimd.collective_compute(
                                "AllGather", mybir.AluOpType.bypass,
                                replica_groups=rg,
                                ins=[xl_loc[l].ap()[NHALF:NPAD, :].opt()],
                                outs=[xl_fullB[l].ap().opt()])

                # ---------------- edge phase ----------------
                with (
                    tc.tile_pool(name=f"g{l}", bufs=3 if l < 2 else 2) as gpool,
                    tc.tile_pool(name=f"ew{l}", bufs=2) as epool,
                    tc.tile_pool(name=f"es{l}", bufs=3) as spool,
                    tc.tile_pool(name=f"am{l}",
                                 bufs=(3 if l < 2 else TMAX + 1)) as ampool,
                    tc.tile_pool(name=f"xw{l}", bufs=2) as xwpool,
                    tc.tile_pool(name=f"psZ{l}", bufs=2, space="PSUM") as psZ,
                    tc.tile_pool(name=f"psE{l}", bufs=1, space="PSUM") as psE,
                    tc.tile_pool(name=f"psD{l}", bufs=1, space="PSUM") as psD,
                    tc.tile_pool(name=f"psP{l}", bufs=1, space="PSUM") as psPool,
                ):
                    if l == 2:
                        pool_ps = [psPool.tile([128, N_GRAPHS], dt.float32,
                                               tag=f"poolps{b}",
                                               name=f"poolps{b}")
                                   for b in range(2)]

                    for w in range(NWIN):
                        TA, TB = TwA[w], TwB[w]
                        T = TA + TB
                        S = T * 128
                        mask_t = epool.tile([128, TMAX * 128], dt.bfloat16,
                                            tag="emask")
                        nc.sync.dma_start(
                            out=mask_t[:, :S],
                            in_=emask_in[:, Soff[w]:Soff[w + 1]])
                        maskT_t = epool.tile([128, TMAX * 128], dt.bfloat16,
                                             tag="emaskT")
                        nc.sync.dma_start(
                            out=maskT_t[:, :S],
                            in_=emaskT_in[:, Soff[w]:Soff[w + 1]])
                        xr_w = xwpool.tile([128, F], dt.bfloat16, tag="xrw")
                        nc.sync.dma_start(
                            out=xr_w[:, :],
                            in_=xr_loc[l][w * 128:(w + 1) * 128, :])
                        gx = gpool.tile([128, TMAX, F], dt.bfloat16, tag="gx")
                        nc.gpsimd.dma_gather(
                            gx[:, :TA, :], xl_fullA[l][:, :],
                            isrcA_t[:, SoffA[w] // 16:SoffA[w + 1] // 16],
                            num_idxs=TA * 128, num_idxs_reg=TA * 128,
                            elem_size=F)
                        nc.gpsimd.dma_gather(
                            gx[:, TA:T, :], xl_fullB[l][:, :],
                            isrcB_t[:, SoffB[w] // 16:SoffB[w + 1] // 16],
                            num_idxs=TB * 128, num_idxs_reg=TB * 128,
                            elem_size=F)

                        ex_w = spool.tile([128, TMAX, H], dt.float32,
                                          tag="exw")
                        exb_w = spool.tile([128, TMAX, H], dt.bfloat16,
                                           tag="exbw")
                        ps_den = psD.tile([128, H], dt.float32, tag="den")
                        NAGG = H if concat else H // 2
                        ps_g = [psE.tile([128, 256], dt.float32,
                                         tag=f"agg{j}", name=f"agg{j}")
                                for j in range(NAGG)]

                        aM_w = []
                        for t in range(T):
                            # ---- s = prelu(xl[src] + xr[dst]) ----
                            s_t = spool.tile([128, F], dt.bfloat16, tag="s")
                            for ch in range(NCH):
                                ps_zc = psZ.tile([128, 512], dt.float32,
                                                 tag="z")
                                nc.tensor.matmul(
                                    ps_zc[:, :],
                                    maskT_t[:, t * 128:(t + 1) * 128],
                                    xr_w[:, ch * 512:(ch + 1) * 512],
                                    start=True, stop=False)
                                nc.tensor.matmul(
                                    ps_zc[:, :],
                                    ident_t[:, :],
                                    gx[:, t, ch * 512:(ch + 1) * 512],
                                    start=False, stop=True)
                                nc.scalar.activation(
                                    s_t[:, ch * 512:(ch + 1) * 512],
                                    ps_zc[:, :], AF.Prelu, alpha=SLOPE)

                            # ---- scores + exp ----
                            sc_t = spool.tile([128, H], dt.float32, tag="sc")
                            if SIM_SAFE or not STT_SCORES:
                                tr = spool.tile([128, F], dt.bfloat16,
                                                tag="trash")
                                nc.vector.tensor_tensor(
                                    tr[:, :], s_t[:, :], att_t[:, :], OP.mult)
                                nc.vector.tensor_reduce(
                                    sc_t[:, :],
                                    tr[:, :].rearrange("p (h c) -> p h c",
                                                       h=H),
                                    mybir.AxisListType.X, OP.add)
                            else:
                                for h in range(H):
                                    tr = spool.tile([128, 256], dt.bfloat16,
                                                    tag="trash")
                                    nc.vector.scalar_tensor_tensor(
                                        out=tr[:, :],
                                        in0=s_t[:, h * 256:(h + 1) * 256],
                                        scalar=1.0,
                                        in1=att_t[:, h * 256:(h + 1) * 256],
                                        op0=OP.mult, op1=OP.mult,
                                        accum_out=sc_t[:, h:h + 1])
                            nc.scalar.activation(
                                ex_w[:, t, :], sc_t[:, :], AF.Exp)
                            nc.scalar.copy(exb_w[:, t, :], ex_w[:, t, :])

                            # ---- alphaM = emask * ex ----
                            aM = ampool.tile([128, H, 128], dt.bfloat16,
                                             tag="aM")
                            aM_w.append(aM)
                            if BCAST_ALPHAM:
                                em_b = (mask_t[:, t * 128:(t + 1) * 128]
                                        .unsqueeze(1)
                                        .broadcast_to([128, H, 128]))
                                ex_b = (exb_w[:, t, :].unsqueeze(2)
                                        .broadcast_to([128, H, 128]))
                                nc.vector.tensor_tensor(
                                    aM[:, :, :], em_b, ex_b, OP.mult)
                            else:
                                for h in range(H):
                                    nc.vector.tensor_scalar(
                                        aM[:, h, :],
                                        mask_t[:, t * 128:(t + 1) * 128],
                                        ex_w[:, t, h:h + 1], None, OP.mult)

                            # ---- denominator + aggregation ----
                            nc.tensor.matmul(
                                ps_den[:, :],
                                mask_t[:, t * 128:(t + 1) * 128],
                                exb_w[:, t, :], start=(t == 0),
                                stop=(t == T - 1))
                            if concat:
                                for h in range(H):
                                    nc.tensor.matmul(
                                        ps_g[h][:, :],
                                        aM[:, h, :],
                                        gx[:, t, h * 256:(h + 1) * 256],
                                        start=(t == 0), stop=(t == T - 1))

                        # ---- window epilogue ----
                        den_t = spool.tile([128, H], dt.float32, tag="wden")
                        nc.vector.tensor_scalar(den_t[:, :], ps_den[:, :H],
                                                float(EPS), None, OP.add)
                        rec_t = spool.tile([128, H], dt.float32, tag="wrec")
                        nc.vector.reciprocal(rec_t[:, :], den_t[:, :])
                        if concat:
                            hn = spool.tile([128, F], dt.bfloat16, tag="hn")
                            for h in range(H):
                                nc.scalar.activation(
                                    hn[:, h * 256:(h + 1) * 256],
                                    ps_g[h][:, :], AF.Copy,
                                    scale=rec_t[:, h:h + 1])
                            nc.vector.tensor_tensor(hn[:, :], hn[:, :],
                                                    bias_t[:, :], OP.add)
                            # elu: max(x, exp(min(x,0)) - 1)
                            mm = spool.tile([128, F], dt.bfloat16,
                                            tag="elu_m")
                            nc.vector.tensor_tensor(mm[:, :], hn[:, :],
                                                    zeros_t[:, :F], OP.min)
                            nc.scalar.activation(mm[:, :], mm[:, :], AF.Exp)
                            hb = spool.tile([128, F], dt.bfloat16, tag="hb")
                            nc.vector.scalar_tensor_tensor(
                                hb[:, :], mm[:, :], -1.0, hn[:, :],
                                OP.add, OP.max)
                            nc.sync.dma_start(
                                out=h_dram[l][w * 128:(w + 1) * 128, :],
                                in_=hb[:, :])
                        else:
                            # mean over heads; two 3-head agg passes
                            rec6 = spool.tile([128, H], dt.float32,
                                              tag="rec6")
                            nc.vector.tensor_scalar(rec6[:, :], rec_t[:, :],
                                                    1.0 / H, None, OP.mult)
                            acc = spool.tile([128, 256], dt.float32,
                                             tag="acc")
                            for gi, grp in enumerate(((0, 1, 2), (3, 4, 5))):
                                for t in range(T):
                                    for j, h in enumerate(grp):
                                        nc.tensor.matmul(
                                            ps_g[j][:, :],
                                            aM_w[t][:, h, :],
                                            gx[:, t, h * 256:(h + 1) * 256],
                                            start=(t == 0), stop=(t == T - 1))
                                for j, h in enumerate(grp):
                                    if h == 0:
                                        nc.vector.tensor_scalar(
                                            acc[:, :], ps_g[j][:, :],
                                            rec6[:, 0:1], None, OP.mult)
                                    else:
                                        nc.vector.scalar_tensor_tensor(
                                            acc[:, :], ps_g[j][:, :],
                                            rec6[:, h:h + 1], acc[:, :],
                                            OP.mult, OP.add)
                            nc.vector.tensor_tensor(acc[:, :], acc[:, :],
                                                    bias_t[:, :], OP.add)
                            # l2 normalize rows
                            ss = spool.tile([128, 1], dt.float32, tag="ss")
                            trash2 = spool.tile([128, 256], dt.float32,
                                                tag="trash2")
                            if SIM_SAFE:
                                nc.vector.tensor_tensor(
                                    trash2[:, :], acc[:, :], acc[:, :],
                                    OP.mult)
                                nc.vector.tensor_reduce(
                                    ss[:, :], trash2[:, :],
                                    mybir.AxisListType.X, OP.add)
                            else:
                                nc.vector.scalar_tensor_tensor(
                                    trash2[:, :], acc[:, :], 1.0, acc[:, :],
                                    OP.mult, OP.mult, accum_out=ss[:, :])
                            nrm = spool.tile([128, 1], dt.float32, tag="nrm")
                            nc.scalar.activation(nrm[:, :], ss[:, :], AF.Sqrt)
                            nc.vector.tensor_scalar(nrm[:, :], nrm[:, :],
                                                    1e-12, None, OP.max)
                            rn = spool.tile([128, 1], dt.float32, tag="rn")
                            nc.vector.reciprocal(rn[:, :], nrm[:, :])
                            hb = spool.tile([128, 256], dt.bfloat16, tag="hb")
                            nc.vector.tensor_scalar(hb[:, :], acc[:, :],
                                                    rn[:, :], None, OP.mult)
                            for b in range(2):
                                nc.tensor.matmul(
                                    pool_ps[b][:, :],
                                    hb[:, b * 128:(b + 1) * 128],
                                    pmask_t[:, w * N_GRAPHS:
                                            (w + 1) * N_GRAPHS],
                                    start=(w == 0), stop=(w == NWIN - 1))

                    if l == 2:
                        for b in range(2):
                            pl = auxpool.tile([128, N_GRAPHS], dt.float32,
                                              tag="pl")
                            nc.vector.tensor_copy(pl[:, :], pool_ps[b][:, :])
                            nc.sync.dma_start(
                                out=pool_loc[b * 128:(b + 1) * 128, :],
                                in_=pl[:, :])

                aux_cm.__exit__(None, None, None)

            # ---- D: pooled -> AllReduce -> MLP ----
            with (
                tc.tile_pool(name="mlp", bufs=1) as mpool,
                tc.tile_pool(name="psM", bufs=1, space="PSUM") as psM,
            ):
                nc.gpsimd.collective_compute(
                    "AllReduce", mybir.AluOpType.add, replica_groups=rg,
                    ins=[pool_loc.ap().opt()],
                    outs=[pool_full.ap().opt()])

                rcnt_t = mpool.tile([128, N_GRAPHS], dt.float32)
                nc.sync.dma_start(out=rcnt_t[:, :], in_=rcnt_in[:, :])
                pz = mpool.tile([128, 2, N_GRAPHS], dt.bfloat16)
                for b in range(2):
                    pf = mpool.tile([128, N_GRAPHS], dt.float32, tag="pf")
                    nc.sync.dma_start(out=pf[:, :],
                                      in_=pool_full[b * 128:(b + 1) * 128, :])
                    nc.vector.tensor_tensor(pz[:, b, :], pf[:, :],
                                            rcnt_t[:, :], OP.mult)

                wfc1_t = mpool.tile([128, 2, 256], dt.bfloat16)
                wfc2_t = mpool.tile([128, 2, 768], dt.bfloat16)
                for b in range(2):
                    nc.sync.dma_start(out=wfc1_t[:, b, :],
                                      in_=wfc1_in[:, b * 256:(b + 1) * 256])
                    nc.sync.dma_start(out=wfc2_t[:, b, :],
                                      in_=wfc2_in[:, b * 768:(b + 1) * 768])
                bfc1_t = mpool.tile([128, 2], dt.float32)
                nc.sync.dma_start(out=bfc1_t[:, :], in_=bfc1_in[:, :])
                bfc2_t = mpool.tile([128, 768], dt.float32)
                nc.sync.dma_start(out=bfc2_t[:, :], in_=bfc2_in[:, :])

                z1 = mpool.tile([128, 2, N_GRAPHS], dt.bfloat16)
                for it in range(2):
                    ps1 = psM.tile([128, N_GRAPHS], dt.float32, tag="ps1")
                    for b in range(2):
                        nc.tensor.matmul(
                            ps1[:, :],
                            wfc1_t[:, b, it * 128:(it + 1) * 128],
                            pz[:, b, :], start=(b == 0), stop=(b == 1))
                    nc.scalar.activation(z1[:, it, :], ps1[:, :], AF.Relu,
                                         bias=bfc1_t[:, it:it + 1], scale=1.0)

                for gt in range(N_GRAPHS // 128):
                    ps2 = psM.tile([128, 768], dt.float32, tag="ps2")
                    for jc, (j0, jw) in enumerate(((0, 512), (512, 256))):
                        for b in range(2):
                            nc.tensor.matmul(
                                ps2[:, j0:j0 + jw],
                                z1[:, b, gt * 128:(gt + 1) * 128],
                                wfc2_t[:, b, j0:j0 + jw],
                                start=(b == 0), stop=(b == 1))
                    zo = mpool.tile([128, 768], dt.float32, tag="zo")
                    nc.vector.tensor_tensor(zo[:, :], ps2[:, :],
                                            bfc2_t[:, :], OP.add)
                    nc.sync.dma_start(
                        out=out_ext[gt * 128:(gt + 1) * 128, :], in_=zo[:, :])

    nc.compile()
    return nc


def kernel(**inputs):
    key, in_maps = _preprocess(inputs)
    if key not in _PROG_CACHE:
        _PROG_CACHE[key] = _build(key)
    nc = _PROG_CACHE[key]
    r = run_bass_kernel_spmd(nc, in_maps, list(range(NCORES)), trace=False)
    return r.results[0]["out"]


# revision 38
# speedup vs baseline: 1.1103x; 1.1103x over previous
"""GATv2 (3-layer, PyG semantics) + global mean pool + MLP on 8 trn2 NeuronCores.

Self-contained: hardcodes problem shapes from nn_GAT_47906065219807.
Sharding: data-parallel over contiguous node ranges (2500 nodes/core); each
core owns edges whose dst lands in its range (edges cross shards via an
AllGather of the source-side projections).

v5:
  - gr expanded from the xr window tile by a PE matmul with a transposed
    edge mask; gx added in-PSUM via an identity matmul; ACT applies Prelu
    straight from PSUM (no DVE add, no gr gather).
  - softmax weights folded into the aggregation matmul's stationary mask
    (alphaM = emask * ex, one broadcast DVE op per tile); division by the
    softmax denominator happens per-node after aggregation.
  - att-dot via fused per-head scalar_tensor_tensor accumulate.
  - each layer's AllGather is split into two half-node collectives: AG-A
    fires halfway through the xl projection and AG-B right after it, so
    both overlap the remaining projection work. Edges are bucketed per
    (window, src-half) so each window runs two gathers, one per half.
"""
import numpy as np
import ml_dtypes

import concourse.bacc as bacc
import concourse.mybir as mybir
import concourse.tile as tile
from concourse.bass_utils import run_bass_kernel_spmd

# problem constants
N_NODES = 20000
N_EDGES = 120000
N_GRAPHS = 512
F_IN = 300
NHID = 256
NOUT = 768
SLOPE = 0.2
EPS = 1e-16

NCORES = 8
NLOC = N_NODES // NCORES            # 2500
NPAD = 2560                         # 20 tiles of 128
NWIN = NPAD // 128                  # 20 windows / node tiles per core
KIN_PAD = 384                       # F_IN padded to 3*128
NHALF = NPAD // 2                   # 1280 rows per AllGather half

# per-layer dims: (K_in_padded, F_out, heads, concat)
LAYERS = [
    (KIN_PAD, 4 * NHID, 4, True),
    (4 * NHID, 4 * NHID, 4, True),
    (4 * NHID, 6 * NHID, 6, False),
]

_BF = ml_dtypes.bfloat16
_PROG_CACHE = {}
SIM_SAFE = False      # sim lacks the fused DVE-accumulator writeback
STT_SCORES = True     # fused per-head stt+accum scores (else mult+3D-reduce)
BCAST_ALPHAM = True   # single broadcast-TT alphaM per tile (else per-head)


def _bf16(a):
    return np.ascontiguousarray(a.astype(_BF)).view(np.uint16)


def _wrap_idx(flat_idx):
    """dma_gather index layout: slot i -> [i % 16, i // 16], replicated to
    128 partitions (8 Q7 cores x 16)."""
    n = flat_idx.shape[0]
    assert n % 16 == 0
    w = flat_idx.reshape(n // 16, 16).T.astype(np.int16)
    return np.tile(w, (8, 1)).copy()


def _preprocess(inputs):
    """Host-side sharding/layout. Returns (key, in_maps)."""
    x = np.asarray(inputs["x"], np.float32)
    ei = np.asarray(inputs["edge_index"]).astype(np.int64)
    batch = np.asarray(inputs["batch"]).astype(np.int64)

    loops = np.arange(N_NODES, dtype=np.int64)
    src = np.concatenate([ei[0], loops])
    dst = np.concatenate([ei[1], loops])

    src_loc = src % NLOC
    src_core = src // NLOC
    src_half = (src_loc >= NHALF).astype(np.int64)          # 0 = A, 1 = B
    # row index within the half's AllGather output [NCORES*NHALF, F]
    src_row = src_core * NHALF + (src_loc - src_half * NHALF)

    core_of = dst // NLOC
    # per (core, window, half) edge buckets
    buckets = [[[[], []] for _ in range(NWIN)] for _ in range(NCORES)]
    order = np.argsort(dst, kind="stable")
    for e in order:
        c = core_of[e]
        dl = dst[e] - c * NLOC
        buckets[c][dl // 128][src_half[e]].append(e)
    TwA = tuple(
        max((len(buckets[c][w][0]) + 127) // 128 for c in range(NCORES))
        for w in range(NWIN)
    )
    TwB = tuple(
        max((len(buckets[c][w][1]) + 127) // 128 for c in range(NCORES))
        for w in range(NWIN)
    )
    # concatenated slot space per window: A tiles then B tiles
    Tw = tuple(a + b for a, b in zip(TwA, TwB))
    Soff = np.concatenate([[0], np.cumsum([t * 128 for t in Tw])])
    SoffA = np.concatenate([[0], np.cumsum([t * 128 for t in TwA])])
    SoffB = np.concatenate([[0], np.cumsum([t * 128 for t in TwB])])

    cnt = np.bincount(batch, minlength=N_GRAPHS).astype(np.float32)
    rcnt = 1.0 / np.maximum(cnt, 1.0)

    def wT_pad(w, kpad):
        wt = w.T.astype(np.float32)                    # [K, F]
        K, F = wt.shape
        out = np.zeros((kpad, F), np.float32)
        out[:K] = wt
        return _bf16(out.reshape(kpad // 128, 128, F).transpose(1, 0, 2)
                     .reshape(128, (kpad // 128) * F))

    shared = {
        "w1l": wT_pad(np.asarray(inputs["c1_wl"]), KIN_PAD),
        "w1r": wT_pad(np.asarray(inputs["c1_wr"]), KIN_PAD),
        "w2l": wT_pad(np.asarray(inputs["c2_wl"]), 4 * NHID),
        "w2r": wT_pad(np.asarray(inputs["c2_wr"]), 4 * NHID),
        "w3l": wT_pad(np.asarray(inputs["c3_wl"]), 4 * NHID),
        "w3r": wT_pad(np.asarray(inputs["c3_wr"]), 4 * NHID),
        "att1": _bf16(np.tile(np.asarray(inputs["c1_att"]).reshape(1, -1), (128, 1))),
        "att2": _bf16(np.tile(np.asarray(inputs["c2_att"]).reshape(1, -1), (128, 1))),
        "att3": _bf16(np.tile(np.asarray(inputs["c3_att"]).reshape(1, -1), (128, 1))),
        "b1": np.tile(np.asarray(inputs["c1_b"], np.float32).reshape(1, -1), (128, 1)),
        "b2": np.tile(np.asarray(inputs["c2_b"], np.float32).reshape(1, -1), (128, 1)),
        "b3": np.tile(np.asarray(inputs["c3_b"], np.float32).reshape(1, -1), (128, 1)),
        "rcnt": np.tile(rcnt.reshape(1, -1), (128, 1)).astype(np.float32),
        "wfc1": wT_pad(np.asarray(inputs["fp1_w"]), 256),
        "wfc2": wT_pad(np.asarray(inputs["fp2_w"]), 256),
        "bfc1": np.asarray(inputs["fp1_b"], np.float32).reshape(2, 128).T.copy(),
        "bfc2": np.tile(np.asarray(inputs["fp2_b"], np.float32).reshape(1, -1),
                        (128, 1)),
        "ident": _bf16(np.eye(128, dtype=np.float32)),
    }

    in_maps = []
    for c in range(NCORES):
        xc = np.zeros((NPAD, KIN_PAD), np.float32)
        xc[:NLOC, :F_IN] = x[c * NLOC:(c + 1) * NLOC]
        xT = xc.T.reshape(KIN_PAD // 128, 128, NPAD).transpose(1, 0, 2)
        xT = _bf16(xT.reshape(128, (KIN_PAD // 128) * NPAD))

        tot = int(Soff[-1])
        isrcA = np.zeros(int(SoffA[-1]), np.int64)
        isrcB = np.zeros(int(SoffB[-1]), np.int64)
        emask = np.zeros((128, tot), np.float32)
        emaskT = np.zeros((128, tot), np.float32)
        for w in range(NWIN):
            s0 = int(Soff[w])
            for hf, (isrc_h, Soff_h, toff) in enumerate(
                    ((isrcA, SoffA, 0), (isrcB, SoffB, TwA[w]))):
            # half A occupies tiles [0, TwA), half B tiles [TwA, TwA+TwB)
                es = buckets[c][w][hf]
                sh0 = int(Soff_h[w])
                for i, e in enumerate(es):
                    isrc_h[sh0 + i] = src_row[e]
                    n = (dst[e] - c * NLOC) - w * 128
                    col = s0 + (toff + i // 128) * 128
                    emask[i % 128, col + n] = 1.0
                    emaskT[n, col + (i % 128)] = 1.0
        pmask = np.zeros((128, NWIN * N_GRAPHS), np.float32)
        bl = batch[c * NLOC:(c + 1) * NLOC]
        for nl in range(NLOC):
            pmask[nl % 128, (nl // 128) * N_GRAPHS + bl[nl]] = 1.0

        m = dict(shared)
        m["xT"] = xT
        m["isrcA"] = _wrap_idx(isrcA)
        m["isrcB"] = _wrap_idx(isrcB)
        m["emask"] = _bf16(emask)
        m["emaskT"] = _bf16(emaskT)
        m["pmask"] = _bf16(pmask)
        in_maps.append(m)
    return (TwA, TwB), in_maps


def _build(key):
    TwA, TwB = key
    Tw = tuple(a + b for a, b in zip(TwA, TwB))
    TMAX = max(Tw)
    Soff = [0]
    SoffA = [0]
    SoffB = [0]
    for a, b in zip(TwA, TwB):
        Soff.append(Soff[-1] + (a + b) * 128)
        SoffA.append(SoffA[-1] + a * 128)
        SoffB.append(SoffB[-1] + b * 128)
    TOT, TOTA, TOTB = Soff[-1], SoffA[-1], SoffB[-1]
    nc = bacc.Bacc("TRN2", target_bir_lowering=False, debug=False,
                   num_devices=NCORES)
    dt = mybir.dt
    AF = mybir.ActivationFunctionType
    OP = mybir.AluOpType

    def inp(name, shape, d):
        return nc.dram_tensor(name, shape, d, kind="ExternalInput")

    xT_in = inp("xT", [128, (KIN_PAD // 128) * NPAD], dt.bfloat16)
    isrcA_in = inp("isrcA", [128, TOTA // 16], dt.int16)
    isrcB_in = inp("isrcB", [128, TOTB // 16], dt.int16)
    emask_in = inp("emask", [128, TOT], dt.bfloat16)
    emaskT_in = inp("emaskT", [128, TOT], dt.bfloat16)
    pmask_in = inp("pmask", [128, NWIN * N_GRAPHS], dt.bfloat16)
    ident_in = inp("ident", [128, 128], dt.bfloat16)
    w_in = [(inp("w1l", [128, 3 * 1024], dt.bfloat16),
             inp("w1r", [128, 3 * 1024], dt.bfloat16)),
            (inp("w2l", [128, 8 * 1024], dt.bfloat16),
             inp("w2r", [128, 8 * 1024], dt.bfloat16)),
            (inp("w3l", [128, 8 * 1536], dt.bfloat16),
             inp("w3r", [128, 8 * 1536], dt.bfloat16))]
    att_in = [inp("att1", [128, 1024], dt.bfloat16),
              inp("att2", [128, 1024], dt.bfloat16),
              inp("att3", [128, 1536], dt.bfloat16)]
    b_in = [inp("b1", [128, 1024], dt.float32),
            inp("b2", [128, 1024], dt.float32),
            inp("b3", [128, 256], dt.float32)]
    rcnt_in = inp("rcnt", [128, N_GRAPHS], dt.float32)
    wfc1_in = inp("wfc1", [128, 2 * 256], dt.bfloat16)
    wfc2_in = inp("wfc2", [128, 2 * 768], dt.bfloat16)
    bfc1_in = inp("bfc1", [128, 2], dt.float32)
    bfc2_in = inp("bfc2", [128, 768], dt.float32)
    out_ext = nc.dram_tensor("out", [N_GRAPHS, NOUT], dt.float32,
                             kind="ExternalOutput")

    # internal DRAM
    xl_loc = [nc.dram_tensor(f"xl_loc{l}", [NPAD, F], dt.bfloat16)
              for l, (_, F, _, _) in enumerate(LAYERS)]
    xr_loc = [nc.dram_tensor(f"xr_loc{l}", [NPAD, F], dt.bfloat16)
              for l, (_, F, _, _) in enumerate(LAYERS)]
    xl_fullA = [nc.dram_tensor(f"xl_fullA{l}", [NCORES * NHALF, F],
                               dt.bfloat16, addr_space="Shared")
                for l, (_, F, _, _) in enumerate(LAYERS)]
    xl_fullB = [nc.dram_tensor(f"xl_fullB{l}", [NCORES * NHALF, F],
                               dt.bfloat16, addr_space="Shared")
                for l, (_, F, _, _) in enumerate(LAYERS)]
    h_dram = [nc.dram_tensor(f"h_dram{l}", [NPAD, 1024], dt.bfloat16)
              for l in range(2)]
    pool_loc = nc.dram_tensor("pool_loc", [256, N_GRAPHS], dt.float32)
    pool_full = nc.dram_tensor("pool_full", [256, N_GRAPHS], dt.float32,
                               addr_space="Shared")

    rg = [list(range(NCORES))]

    with tile.TileContext(nc) as tc:
        with (
            tc.tile_pool(name="persist", bufs=1) as ppool,
        ):
            isrcA_t = ppool.tile([128, TOTA // 16], dt.int16)
            nc.sync.dma_start(out=isrcA_t[:, :], in_=isrcA_in[:, :])
            isrcB_t = ppool.tile([128, TOTB // 16], dt.int16)
            nc.sync.dma_start(out=isrcB_t[:, :], in_=isrcB_in[:, :])
            ident_t = ppool.tile([128, 128], dt.bfloat16)
            nc.sync.dma_start(out=ident_t[:, :], in_=ident_in[:, :])
            zeros_t = ppool.tile([128, 1024], dt.bfloat16)
            nc.vector.memset(zeros_t[:, :], 0.0)

            for l, (K, F, H, concat) in enumerate(LAYERS):
                KB = K // 128
                NCH = F // 512

                # ---------------- projections ----------------
                with (
                    tc.tile_pool(name=f"pj{l}", bufs=1) as pjpool,
                    tc.tile_pool(name=f"mm{l}", bufs=4) as mmpool,
                    tc.tile_pool(name=f"psA{l}", bufs=2, space="PSUM") as psA,
                ):
                    hT = pjpool.tile([128, KB, NPAD], dt.bfloat16, tag="hT")
                    if l == 0:
                        for b in range(KB):
                            nc.sync.dma_start(
                                out=hT[:, b, :],
                                in_=xT_in[:, b * NPAD:(b + 1) * NPAD])
                    else:
                        QN = NPAD // 4
                        for q in range(4):
                            for b in range(KB):
                                nc.sync.dma_start(
                                    out=hT[:, b, q * QN:(q + 1) * QN],
                                    in_=h_dram[l - 1][q * QN:(q + 1) * QN,
                                                      b * 128:(b + 1) * 128],
                                    transpose=True)
                    wl_t = pjpool.tile([128, KB, F], dt.bfloat16)
                    wr_t = pjpool.tile([128, KB, F], dt.bfloat16)
                    for wt, win in ((wl_t, w_in[l][0]), (wr_t, w_in[l][1])):
                        for b in range(KB):
                            nc.sync.dma_start(out=wt[:, b, :],
                                              in_=win[:, b * F:(b + 1) * F])
                    for side, (wt, dst_dram) in enumerate(
                            ((wl_t, xl_loc[l]), (wr_t, xr_loc[l]))):
                        for t in range(NWIN):
                            for ch in range(NCH):
                                ps = psA.tile([128, 512], dt.float32,
                                              tag="mmps")
                                for b in range(KB):
                                    nc.tensor.matmul(
                                        ps[:, :],
                                        hT[:, b, t * 128:(t + 1) * 128],
                                        wt[:, b, ch * 512:(ch + 1) * 512],
                                        start=(b == 0), stop=(b == KB - 1))
                                ob = mmpool.tile([128, 512], dt.bfloat16,
                                                 tag="mmout")
                                nc.scalar.copy(ob[:, :], ps[:, :])
                                nc.sync.dma_start(
                                    out=dst_dram[t * 128:(t + 1) * 128,
                                                 ch * 512:(ch + 1) * 512],
                                    in_=ob[:, :])
                            if side == 0 and t == NWIN // 2 - 1:
                                nc.gpsimd.collective_compute(
                                    "AllGather", mybir.AluOpType.bypass,
                                    replica_groups=rg,
                                    ins=[xl_loc[l].ap()[0:NHALF, :].opt()],
                                    outs=[xl_fullA[l].ap().opt()])
                        if side == 0:
                            nc.gpsimd.collective_compute(
                                "AllGather", mybir.AluOpType.bypass,
                                replica_groups=rg,
                                ins=[xl_loc[l].ap()[NHALF:NPAD, :].opt()],
                                outs=[xl_fullB[l].ap().opt()])

                # ---------------- edge phase ----------------
                with (
                    tc.tile_pool(name=f"g{l}", bufs=3 if l < 2 else 2) as gpool,
                    tc.tile_pool(name=f"ew{l}", bufs=2) as epool,
                    tc.tile_pool(name=f"es{l}", bufs=3) as spool,
                    tc.tile_pool(name=f"am{l}",
                                 bufs=(3 if l < 2 else TMAX + 1)) as ampool,
                    tc.tile_pool(name=f"xw{l}", bufs=2) as xwpool,
                    tc.tile_pool(name=f"psZ{l}", bufs=2, space="PSUM") as psZ,
                    tc.tile_pool(name=f"psE{l}", bufs=1, space="PSUM") as psE,
                    tc.tile_pool(name=f"psD{l}", bufs=1, space="PSUM") as psD,
                    tc.tile_pool(name=f"psP{l}", bufs=1, space="PSUM") as psPool,
                    tc.tile_pool(name=f"aux{l}", bufs=1) as auxpool,
                ):
                    att_t = auxpool.tile([128, F], dt.bfloat16)
                    nc.sync.dma_start(out=att_t[:, :], in_=att_in[l][:, :])
                    bias_t = auxpool.tile([128, F if concat else 256],
                                          dt.float32)
                    nc.sync.dma_start(out=bias_t[:, :], in_=b_in[l][:, :])
                    if l == 2:
                        pmask_t = auxpool.tile([128, NWIN * N_GRAPHS],
                                               dt.bfloat16)
                        nc.sync.dma_start(out=pmask_t[:, :], in_=pmask_in[:, :])
                        pool_ps = [psPool.tile([128, N_GRAPHS], dt.float32,
                                               tag=f"poolps{b}",
                                               name=f"poolps{b}")
                                   for b in range(2)]

                    for w in range(NWIN):
                        TA, TB = TwA[w], TwB[w]
                        T = TA + TB
                        S = T * 128
                        mask_t = epool.tile([128, TMAX * 128], dt.bfloat16,
                                            tag="emask")
                        nc.sync.dma_start(
                            out=mask_t[:, :S],
                            in_=emask_in[:, Soff[w]:Soff[w + 1]])
                        maskT_t = epool.tile([128, TMAX * 128], dt.bfloat16,
                                             tag="emaskT")
                        nc.sync.dma_start(
                            out=maskT_t[:, :S],
                            in_=emaskT_in[:, Soff[w]:Soff[w + 1]])
                        xr_w = xwpool.tile([128, F], dt.bfloat16, tag="xrw")
                        nc.sync.dma_start(
                            out=xr_w[:, :],
                            in_=xr_loc[l][w * 128:(w + 1) * 128, :])
                        gx = gpool.tile([128, TMAX, F], dt.bfloat16, tag="gx")
                        nc.gpsimd.dma_gather(
                            gx[:, :TA, :], xl_fullA[l][:, :],
                            isrcA_t[:, SoffA[w] // 16:SoffA[w + 1] // 16],
                            num_idxs=TA * 128, num_idxs_reg=TA * 128,
                            elem_size=F)
                        nc.gpsimd.dma_gather(
                            gx[:, TA:T, :], xl_fullB[l][:, :],
                            isrcB_t[:, SoffB[w] // 16:SoffB[w + 1] // 16],
                            num_idxs=TB * 128, num_idxs_reg=TB * 128,
                            elem_size=F)

                        ex_w = spool.tile([128, TMAX, H], dt.float32,
                                          tag="exw")
                        exb_w = spool.tile([128, TMAX, H], dt.bfloat16,
                                           tag="exbw")
                        ps_den = psD.tile([128, H], dt.float32, tag="den")
                        NAGG = H if concat else H // 2
                        ps_g = [psE.tile([128, 256], dt.float32,
                                         tag=f"agg{j}", name=f"agg{j}")
                                for j in range(NAGG)]

                        aM_w = []
                        for t in range(T):
                            # ---- s = prelu(xl[src] + xr[dst]) ----
                            s_t = spool.tile([128, F], dt.bfloat16, tag="s")
                            for ch in range(NCH):
                                ps_zc = psZ.tile([128, 512], dt.float32,
                                                 tag="z")
                                nc.tensor.matmul(
                                    ps_zc[:, :],
                                    maskT_t[:, t * 128:(t + 1) * 128],
                                    xr_w[:, ch * 512:(ch + 1) * 512],
                                    start=True, stop=False)
                                nc.tensor.matmul(
                                    ps_zc[:, :],
                                    ident_t[:, :],
                                    gx[:, t, ch * 512:(ch + 1) * 512],
                                    start=False, stop=True)
                                nc.scalar.activation(
                                    s_t[:, ch * 512:(ch + 1) * 512],
                                    ps_zc[:, :], AF.Prelu, alpha=SLOPE)

                            # ---- scores + exp ----
                            sc_t = spool.tile([128, H], dt.float32, tag="sc")
                            if SIM_SAFE or not STT_SCORES:
                                tr = spool.tile([128, F], dt.bfloat16,
                                                tag="trash")
                                nc.vector.tensor_tensor(
                                    tr[:, :], s_t[:, :], att_t[:, :], OP.mult)
                                nc.vector.tensor_reduce(
                                    sc_t[:, :],
                                    tr[:, :].rearrange("p (h c) -> p h c",
                                                       h=H),
                                    mybir.AxisListType.X, OP.add)
                            else:
                                for h in range(H):
                                    tr = spool.tile([128, 256], dt.bfloat16,
                                                    tag="trash")
                                    nc.vector.scalar_tensor_tensor(
                                        out=tr[:, :],
                                        in0=s_t[:, h * 256:(h + 1) * 256],
                                        scalar=1.0,
                                        in1=att_t[:, h * 256:(h + 1) * 256],
                                        op0=OP.mult, op1=OP.mult,
                                        accum_out=sc_t[:, h:h + 1])
                            nc.scalar.activation(
                                ex_w[:, t, :], sc_t[:, :], AF.Exp)
                            nc.scalar.copy(exb_w[:, t, :], ex_w[:, t, :])

                            # ---- alphaM = emask * ex ----
                            aM = ampool.tile([128, H, 128], dt.bfloat16,
                                             tag="aM")
                            aM_w.append(aM)
                            if BCAST_ALPHAM:
                                em_b = (mask_t[:, t * 128:(t + 1) * 128]
                                        .unsqueeze(1)
                                        .broadcast_to([128, H, 128]))
                                ex_b = (exb_w[:, t, :].unsqueeze(2)
                                        .broadcast_to([128, H, 128]))
                                nc.vector.tensor_tensor(
                                    aM[:, :, :], em_b, ex_b, OP.mult)
                            else:
                                for h in range(H):
                                    nc.vector.tensor_scalar(
                                        aM[:, h, :],
                                        mask_t[:, t * 128:(t + 1) * 128],
                                        ex_w[:, t, h:h + 1], None, OP.mult)

                            # ---- denominator + aggregation ----
                            nc.tensor.matmul(
                                ps_den[:, :],
                                mask_t[:, t * 128:(t + 1) * 128],
                                exb_w[:, t, :], start=(t == 0),
                                stop=(t == T - 1))
                            if concat:
                                for h in range(H):
                                    nc.tensor.matmul(
                                        ps_g[h][:, :],
                                        aM[:, h, :],
                                        gx[:, t, h * 256:(h + 1) * 256],
                                        start=(t == 0), stop=(t == T - 1))

                        # ---- window epilogue ----
                        den_t = spool.tile([128, H], dt.float32, tag="wden")
                        nc.vector.tensor_scalar(den_t[:, :], ps_den[:, :H],
                                                float(EPS), None, OP.add)
                        rec_t = spool.tile([128, H], dt.float32, tag="wrec")
                        nc.vector.reciprocal(rec_t[:, :], den_t[:, :])
                        if concat:
                            hn = spool.tile([128, F], dt.bfloat16, tag="hn")
                            for h in range(H):
                                nc.scalar.activation(
                                    hn[:, h * 256:(h + 1) * 256],
                                    ps_g[h][:, :], AF.Copy,
                                    scale=rec_t[:, h:h + 1])
                            nc.vector.tensor_tensor(hn[:, :], hn[:, :],
                                                    bias_t[:, :], OP.add)
                            # elu: max(x, exp(min(x,0)) - 1)
                            mm = spool.tile([128, F], dt.bfloat16,
                                            tag="elu_m")
                            nc.vector.tensor_tensor(mm[:, :], hn[:, :],
                                                    zeros_t[:, :F], OP.min)
                            nc.scalar.activation(mm[:, :], mm[:, :], AF.Exp)
                            hb = spool.tile([128, F], dt.bfloat16, tag="hb")
                            nc.vector.scalar_tensor_tensor(
                                hb[:, :], mm[:, :], -1.0, hn[:, :],
                                OP.add, OP.max)
                            nc.sync.dma_start(
                                out=h_dram[l][w * 128:(w + 1) * 128, :],
                                in_=hb[:, :])
                        else:
                            # mean over heads; two 3-head agg passes
                            rec6 = spool.tile([128, H], dt.float32,
                                              tag="rec6")
                            nc.vector.tensor_scalar(rec6[:, :], rec_t[:, :],
                                                    1.0 / H, None, OP.mult)
                            acc = spool.tile([128, 256], dt.float32,
                                             tag="acc")
                            for gi, grp in enumerate(((0, 1, 2), (3, 4, 5))):
                                for t in range(T):
                                    for j, h in enumerate(grp):
                                        nc.tensor.matmul(
                                            ps_g[j][:, :],
                                            aM_w[t][:, h, :],
                                            gx[:, t, h * 256:(h + 1) * 256],
                                            start=(t == 0), stop=(t == T - 1))
                                for j, h in enumerate(grp):
                                    if h == 0:
                                        nc.vector.tensor_scalar(
                                            acc[:, :], ps_g[j][:, :],
                                            rec6[:, 0:1], None, OP.mult)
                                    else:
                                        nc.vector.scalar_tensor_tensor(
                                            acc[:, :], ps_g[j][:, :],
                                            rec6[:, h:h + 1], acc[:, :],
                                            OP.mult, OP.add)
                            nc.vector.tensor_tensor(acc[:, :], acc[:, :],
                                                    bias_t[:, :], OP.add)
                            # l2 normalize rows
                            ss = spool.tile([128, 1], dt.float32, tag="ss")
                            trash2 = spool.tile([128, 256], dt.float32,
                                                tag="trash2")
                            if SIM_SAFE:
                                nc.vector.tensor_tensor(
                                    trash2[:, :], acc[:, :], acc[:, :],
                                    OP.mult)
                                nc.vector.tensor_reduce(
                                    ss[:, :], trash2[:, :],
                                    mybir.AxisListType.X, OP.add)
                            else:
                                nc.vector.scalar_tensor_tensor(
                                    trash2[:, :], acc[:, :], 1.0, acc[:, :],
                                    OP.mult, OP.mult, accum_out=ss[:, :])
                            nrm = spool.tile([128, 1], dt.float32, tag="nrm")
                            nc.scalar.activation(nrm[:, :], ss[:, :], AF.Sqrt)
                            nc.vector.tensor_scalar(nrm[:, :], nrm[:, :],
                                                    1e-12, None, OP.max)
                            rn = spool.tile([128, 1], dt.float32, tag="rn")
                            nc.vector.reciprocal(rn[:, :], nrm[:, :])
                            hb = spool.tile([128, 256], dt.bfloat16, tag="hb")
                            nc.vector.tensor_scalar(hb[:, :], acc[:, :],
                                                    rn[:, :], None, OP.mult)
                            for b in range(2):
                                nc.tensor.matmul(
                                    pool_ps[b][:, :],
                                    hb[:, b * 128:(b + 1) * 128],
                                    pmask_t[:, w * N_GRAPHS:
                                            (w + 1) * N_GRAPHS],
                                    start=(w == 0), stop=(w == NWIN - 1))

                    if l == 2:
                        for b in range(2):
                            pl = auxpool.tile([128, N_GRAPHS], dt.float32,
                                              tag="pl")
                            nc.vector.tensor_copy(pl[:, :], pool_ps[b][:, :])
                            nc.sync.dma_start(
                                out=pool_loc[b * 128:(b + 1) * 128, :],
                                in_=pl[:, :])

            # ---- D: pooled -> AllReduce -> MLP ----
            with (
                tc.tile_pool(name="mlp", bufs=1) as mpool,
                tc.tile_pool(name="psM", bufs=1, space="PSUM") as psM,
            ):
                nc.gpsimd.collective_compute(
                    "AllReduce", mybir.AluOpType.add, replica_groups=rg,
                    ins=[pool_loc.ap().opt()],
                    outs=[pool_full.ap().opt()])

                rcnt_t = mpool.tile([128, N_GRAPHS], dt.float32)
                nc.sync.dma_start(out=rcnt_t[:, :], in_=rcnt_in[:, :])
                pz = mpool.tile([128, 2, N_GRAPHS], dt.bfloat16)
                for b in range(2):
                    pf = mpool.tile([128, N_GRAPHS], dt.float32, tag="pf")
                    nc.sync.dma_start(out=pf[:, :],
                                      in_=pool_full[b * 128:(b + 1) * 128, :])
                    nc.vector.tensor_tensor(pz[:, b, :], pf[:, :],
                                            rcnt_t[:, :], OP.mult)

                wfc1_t = mpool.tile([128, 2, 256], dt.bfloat16)
                wfc2_t = mpool.tile([128, 2, 768], dt.bfloat16)
                for b in range(2):
                    nc.sync.dma_start(out=wfc1_t[:, b, :],
                                      in_=wfc1_in[:, b * 256:(b + 1) * 256])
                    nc.sync.dma_start(out=wfc2_t[:, b, :],
                                      in_=wfc2_in[:, b * 768:(b + 1) * 768])
                bfc1_t = mpool.tile([128, 2], dt.float32)
                nc.sync.dma_start(out=bfc1_t[:, :], in_=bfc1_in[:, :])
                bfc2_t = mpool.tile([128, 768], dt.float32)
                nc.sync.dma_start(out=bfc2_t[:, :], in_=bfc2_in[:, :])

                z1 = mpool.tile([128, 2, N_GRAPHS], dt.bfloat16)
                for it in range(2):
                    ps1 = psM.tile([128, N_GRAPHS], dt.float32, tag="ps1")
                    for b in range(2):
                        nc.tensor.matmul(
                            ps1[:, :],
                            wfc1_t[:, b, it * 128:(it + 1) * 128],
                            pz[:, b, :], start=(b == 0), stop=(b == 1))
                    nc.scalar.activation(z1[:, it, :], ps1[:, :], AF.Relu,
                                         bias=bfc1_t[:, it:it + 1], scale=1.0)

                for gt in range(N_GRAPHS // 128):
                    ps2 = psM.tile([128, 768], dt.float32, tag="ps2")
                    for jc, (j0, jw) in enumerate(((0, 512), (512, 256))):
                        for b in range(2):
                            nc.tensor.matmul(
                                ps2[:, j0:j0 + jw],
                                z1[:, b, gt * 128:(gt + 1) * 128],
                                wfc2_t[:, b, j0:j0 + jw],
                                start=(b == 0), stop=(b == 1))
                    zo = mpool.tile([128, 768], dt.float32, tag="zo")
                    nc.vector.tensor_tensor(zo[:, :], ps2[:, :],
                                            bfc2_t[:, :], OP.add)
                    nc.sync.dma_start(
                        out=out_ext[gt * 128:(gt + 1) * 128, :], in_=zo[:, :])

    nc.compile()
    return nc


def kernel(**inputs):
    key, in_maps = _preprocess(inputs)
    if key not in _PROG_CACHE:
        _PROG_CACHE[key] = _build(key)
    nc = _PROG_CACHE[key]
    r = run_bass_kernel_spmd(nc, in_maps, list(range(NCORES)), trace=False)
    return r.results[0]["out"]
